# revision 20
# baseline (speedup 1.0000x reference)
"""BiAttention TRN2 kernel: data-parallel over batch across 8 NeuronCores.

Self-contained: hardcodes B=32, Tc=2048, Tq=256, D=256, 8 cores, 4 batches/core.

Design:
- Host pre-transposes C and Q: device receives C^T f32 (sim lhsT), C bf16
  (q2c lhsT), Q^T f32 (sim rhs), Q bf16 with a ones column (mm2 rhs).
  No C/Q transposes or PSUM->SBUF staging on PE/DVE.
- Row sums come free from the ones column of the mm2 rhs (out col 256),
  so exp needs no accumulator read.
- q2c computed as 1-row-moving matmuls (out [128,1]) -- near-zero PE cost;
  normalization by the total happens on host.
- Output stored bf16 (halves O DMA); normalize-muls split 3:1 ACT/DVE;
  row-max quad reductions on DVE; all DMAs issued from SP HWDGE queues.
- All engine threads run one continuous software-pipelined stream over the
  64 global blocks (no per-batch barriers); deep rings (pS 6 slots, p_sb 6,
  dual P^T PSUM banks, pO ring 3) keep cross-engine feedback loops slack.
"""
import numpy as np
import ml_dtypes

import concourse.bass as bass
from concourse import mybir
from concourse.bass_utils import run_bass_kernel_spmd

F32 = mybir.dt.float32
F32R = mybir.dt.float32r
BF16 = mybir.dt.bfloat16
Exp = mybir.ActivationFunctionType.Exp
AX = mybir.AxisListType
OP = mybir.AluOpType

B, TC, TQ, D = 32, 2048, 256, 256
NCORES = 8
NB = B // NCORES          # batches per core = 4
NBLK = TC // 128          # c-blocks per batch = 16
NTOT = NB * NBLK          # total blocks = 64
NEG = -(2.0 ** 96)
SQ = 2.0 ** 48
QN_W = TQ + 1             # mm2 rhs width: D cols of Q + ones column

# pipeline stage lags (in global slots)
L_EX = 5
L_PT = 7
L_MM = 10
L_RC = 12
L_OC = 13
NSLOT = NTOT + L_OC + 2


def outcp_on_dve(n):
    return n % 8 in (2, 5, 7)


def cnt_a(m):
    """# of outcp indices 0..m handled by ACT."""
    return sum(1 for j in range(m + 1) if not outcp_on_dve(j))


def cnt_d(m):
    """# of outcp indices 0..m handled by DVE."""
    return sum(1 for j in range(m + 1) if outcp_on_dve(j))


def build_program():
    nc = bass.Bass()
    ctq_d = nc.declare_dram_parameter("ctq", [NB, 2, 128, TQ + TC], F32R,
                                      isOutput=False)
    cn_d = nc.declare_dram_parameter("cn", [NB, TC, D], BF16, isOutput=False)
    qn_d = nc.declare_dram_parameter("qn", [NB, 2, 128, QN_W], BF16, isOutput=False)
    msk_d = nc.declare_dram_parameter("msk", [NB, 2, TC + TQ], F32R, isOutput=False)
    id_d = nc.declare_dram_parameter("identb", [128, 128], BF16, isOutput=False)
    c100_d = nc.declare_dram_parameter("c100", [128, 1], F32, isOutput=False)
    ones_d = nc.declare_dram_parameter("ones128", [128, 1], F32, isOutput=False)

    o_d = nc.declare_dram_parameter("o", [NB, TC, D], BF16, isOutput=True)
    qc_d = nc.declare_dram_parameter("qc", [NB, 128, 3], F32, isOutput=True)

    from contextlib import ExitStack
    es = ExitStack()
    _ctr = [0]

    def sb(shape, dt, name=None):
        _ctr[0] += 1
        return es.enter_context(nc.sbuf_tensor(name or f"sb{_ctr[0]}", shape, dt))

    def ps(shape, dt, name=None):
        _ctr[0] += 1
        return es.enter_context(nc.psum_tensor(name or f"ps{_ctr[0]}", shape, dt))

    def sem(name):
        return es.enter_context(nc.semaphore(name))

    # ---- SBUF ----
    # merged [Q^T | C^T] per batch: cols 0:TQ = Q^T, TQ: = C^T
    ctq = [sb([128, 2, TQ + TC], F32R) for _ in range(3)]
    cbn = [sb([128, NBLK, D], BF16) for _ in range(3)]  # C natural bf16
    qnb = [sb([128, 2, QN_W], BF16) for _ in range(3)]  # Q nat + ones col
    msk = [sb([2, TC + TQ], F32R) for _ in range(3)]    # [c-mask | q-mask] features
    identb = sb([128, 128], BF16)
    c100 = sb([128, 1], F32)                            # bias constant -100
    ones128 = sb([128, 1], F32)
    p_sb = [sb([128, TQ], BF16) for _ in range(6)]      # exp(S-m) (bf16), 6-deep
    ptr = [sb([128, 2, 2, 128], BF16) for _ in range(2)]  # P^T (q, blkpar, qhalf, c)
    NM = [sb([128, NBLK], F32) for _ in range(2)]       # -rowmax per block column
    RS = [sb([128, NBLK], F32) for _ in range(2)]       # 1/rowsum
    E_all = [sb([128, NBLK], BF16) for _ in range(2)]   # exp(m - 100) for q2c
    esum = [sb([128, 1], F32) for _ in range(2)]
    o_sb = [sb([128, NBLK, D], BF16) for _ in range(2)]  # output batch buffer
    qc_sb = [sb([128, 3], F32) for _ in range(2)]       # staged q2cT + total

    # ---- PSUM (8 banks) ----
    pS = ps([128, 6, 256], F32)       # sim ring, 6 slots (3 banks)
    # P^T pair banks: lower half (f32 cols 0:256) holds bf16 P^T pairs via
    # bitcast; upper half of bank 1 doubles as the q2c accumulator region.
    pPT = [ps([128, 512], F32) for _ in range(2)]
    pOb = [ps([128, QN_W], F32) for _ in range(3)]   # mm2 out (+rowsum col)
    pM = pPT[1]                       # q2cT cols 300:302, total at [0:1, 310:311]

    def pO(ko):
        return pOb[ko][:, 0:256]

    def psum_col(n):
        return pOb[n % 3][:, 256:257]

    sems = {}
    for name in ("s_out", "s_qc", "pe_s", "pe_pt", "pe_o", "pt_",
                 "dve_nm", "act_p", "act_oA", "act_oD", "dve_ptr", "dve_rs",
                 "at", "dv_qc"):
        sems[name] = sem(name)
    IN_TAGS = ["msk", "ctq0", "ctq1", "ctq2", "ctq3", "ctq4", "qnb", "cbn",
               "const"]
    s_in = {t: sem("s_" + t) for t in IN_TAGS}
    s_out = sems["s_out"]; s_qc = sems["s_qc"]
    pe_s = sems["pe_s"]; pe_pt = sems["pe_pt"]; pe_o = sems["pe_o"]
    pt_ = sems["pt_"]; dve_nm = sems["dve_nm"]; act_p = sems["act_p"]
    act_oA = sems["act_oA"]; act_oD = sems["act_oD"]
    dve_ptr = sems["dve_ptr"]; dve_rs = sems["dve_rs"]; at = sems["at"]
    dv_qc = sems["dv_qc"]

    # Input DMA schedule: per batch, sim-critical tensors first, C^T in
    # 4 column-quarters so early blocks can start before the full load.
    # Consts are interleaved after batch 0's sim-critical loads.
    CTQ_CUTS = [0, TQ + 128, TQ + 128 * 5, TQ + 128 * 9, TQ + 128 * 13,
                TQ + TC]
    NCHUNK = len(CTQ_CUTS) - 1
    TH_I = {0: 0, 1: 1, 5: 2, 9: 3, 13: 4}   # block -> chunk it needs

    blk = es.enter_context(nc.Block())
    with blk:
        # ---------------- SP: all DMAs ----------------
        @blk.sync
        def _(sy):
            def issue_one(b, tag):
                if tag == "msk":
                    return sy.dma_start(msk[b % 3][:], msk_d[b])
                if tag.startswith("ctq"):
                    q = int(tag[3])
                    lo, hi = CTQ_CUTS[q], CTQ_CUTS[q + 1]
                    return sy.dma_start(
                        ctq[b % 3][:, :, lo:hi],
                        ctq_d[b, :, :, lo:hi].rearrange("k p c -> p k c"))
                if tag == "qnb":
                    return sy.dma_start(qnb[b % 3][:],
                                        qn_d[b].rearrange("k p d -> p k d"))
                if tag == "cbn":
                    return sy.dma_start(
                        cbn[b % 3][:],
                        cn_d[b].rearrange("(i p) d -> p i d", p=128))
                raise AssertionError(tag)

            def issue_inputs(b):
                if b >= 3:
                    # WAR: batch b-3 consumers done with the b%3 buffers
                    sy.wait_ge(pe_s, 16 * (b - 2))
                    sy.wait_ge(pe_o, 16 * (b - 2))
                    sy.wait_ge(pt_, b - 2)
                tags = ["msk"] + [f"ctq{q}" for q in range(NCHUNK)]
                if b == 0:
                    tags += ["consts"]
                tags += ["qnb", "cbn"]
                for tag in tags:
                    if tag == "consts":
                        sy.dma_start(identb[:], id_d[:]).then_inc(s_in["const"], 16)
                        sy.dma_start(c100[:], c100_d[:]).then_inc(s_in["const"], 16)
                        sy.dma_start(ones128[:], ones_d[:]).then_inc(s_in["const"], 16)
                    else:
                        if b >= 1:
                            # serialize same-tag DMAs across batches so tag
                            # sem thresholds are unambiguous under unordered
                            # DMA completion
                            sy.wait_ge(s_in[tag], 16 * b)
                        issue_one(b, tag).then_inc(s_in[tag], 16)

            issue_inputs(0)
            issue_inputs(1)
            for b in range(NB):
                if b + 2 < NB:
                    issue_inputs(b + 2)
                if b >= 2:
                    sy.wait_ge(s_out, 64 * (b - 1))
                for q4 in range(4):
                    m = 16 * b + 4 * q4 + 3
                    sy.wait_ge(act_oA, cnt_a(m))
                    sy.wait_ge(act_oD, cnt_d(m))
                    sy.dma_start(
                        o_d[b, 512 * q4:512 * (q4 + 1)].rearrange(
                            "(i p) d -> p i d", p=128),
                        o_sb[b % 2][:, 4 * q4:4 * (q4 + 1), :]).then_inc(s_out, 16)
                sy.wait_ge(dv_qc, b + 1)
                sy.dma_start(qc_d[b], qc_sb[b % 2][:]).then_inc(s_qc, 16)

        # ---------------- PE ----------------
        @blk.tensor
        def _(t):
            def sim(n):
                b, i = divmod(n, NBLK)
                sl = n % 6
                if i == 0:
                    t.wait_ge(s_in["msk"], 16 * (b + 1))
                if i in TH_I:
                    t.wait_ge(s_in[f"ctq{TH_I[i]}"], 16 * (b + 1))
                if n >= 6:
                    t.wait_ge(act_p, n - 5)   # exp(n-6) done -> pS slot free
                t.matmul(pS[:, sl, :],
                         msk[b % 3][:, TQ + 128 * i:TQ + 128 * (i + 1)],
                         msk[b % 3][:, 0:TQ], start=True, stop=False)
                t.matmul(pS[:, sl, :],
                         ctq[b % 3][:, 0, TQ + 128 * i:TQ + 128 * (i + 1)],
                         ctq[b % 3][:, 0, 0:TQ], start=False, stop=False)
                t.matmul(pS[:, sl, :],
                         ctq[b % 3][:, 1, TQ + 128 * i:TQ + 128 * (i + 1)],
                         ctq[b % 3][:, 1, 0:TQ], start=False,
                         stop=True).then_inc(pe_s, 1)

            def pt_tr(n):
                k = n % 2
                pb = (n // 2) % 2
                if n >= 4:
                    t.wait_ge(dve_ptr, n // 2 - 1)   # pPT[pb] prior pair copied
                if n == 0:
                    t.wait_ge(s_in["const"], 48)
                ptb = pPT[pb][:].bitcast(BF16)
                tr0 = t.transpose(ptb[:, k * 256:k * 256 + 128],
                                  p_sb[n % 6][:, 0:128], identb[:])
                tr0._wait_ge(act_p, n + 1)
                t.transpose(ptb[:, k * 256 + 128:k * 256 + 256],
                            p_sb[n % 6][:, 128:256], identb[:]).then_inc(pe_pt, 1)

            def mm2(n):
                b, i = divmod(n, NBLK)
                ko = n % 3
                pp = (n // 2) % 2
                if i == 0:
                    t.wait_ge(s_in["qnb"], 16 * (b + 1))
                if n >= 3:
                    m = n - 3
                    t.wait_ge(act_oA, cnt_a(m))    # outcp(n-3) done
                    t.wait_ge(act_oD, cnt_d(m))
                    t.wait_ge(dve_rs, n - 2)       # recip(n-3) done
                mm0 = t.matmul(pOb[ko][:], ptr[pp][:, n % 2, 0],
                               qnb[b % 3][:, 0, :], start=True, stop=False)
                mm0._wait_ge(dve_ptr, n // 2 + 1)
                t.matmul(pOb[ko][:], ptr[pp][:, n % 2, 1], qnb[b % 3][:, 1, :],
                         start=False, stop=True).then_inc(pe_o, 1)

            def tail(b):
                t.wait_ge(s_in["cbn"], 16 * (b + 1))
                t.wait_ge(at, b + 1)          # E_all/esum ready
                if b >= 1:
                    t.wait_ge(dv_qc, b)       # qc staging of b-1 done (pM free)
                for dh in range(2):
                    for i in range(NBLK):
                        t.matmul(pM[:, 300 + dh:301 + dh],
                                 cbn[b % 3][:, i, 128 * dh:128 * (dh + 1)],
                                 E_all[b % 2][:, i:i + 1],
                                 start=(i == 0), stop=(i == NBLK - 1))
                t.matmul(pM[0:1, 310:311], esum[b % 2][:], ones128[:],
                         start=True, stop=True).then_inc(pt_, 1)

            for g in range(NSLOT):
                n = g - L_PT
                if 0 <= n < NTOT:
                    pt_tr(n)
                n = g - L_MM
                if 0 <= n < NTOT:
                    mm2(n)
                n = g
                if 0 <= n < NTOT:
                    sim(n)
                for b in range(NB):
                    if g == 16 * b + 23:
                        tail(b)

        # ---------------- ACT ----------------
        @blk.scalar
        def _(s):
            def ex(n):
                b, i = divmod(n, NBLK)
                sl = n % 6
                if n >= 6:
                    s.wait_ge(pe_pt, n - 5)   # p_sb 6-deep WAR
                ac = s.activation(p_sb[n % 6][:], pS[:, sl, :], Exp,
                                  bias=NM[b % 2][:, i:i + 1])
                ac._wait_ge(dve_nm, 8 * b + i // 2 + 1)
                ac.then_inc(act_p, 1)

            def outcp_a(n):
                b, i = divmod(n, NBLK)
                ko = n % 3
                s.wait_ge(dve_rs, n + 1)
                if i == 0 and b >= 2:
                    s.wait_ge(s_out, 64 * (b - 1))
                s.mul(o_sb[b % 2][:, i, :], pO(ko),
                      RS[b % 2][:, i:i + 1]).then_inc(act_oA, 1)

            def t1(b):
                if b == 0:
                    s.wait_ge(s_in["const"], 48)
                s.wait_ge(dve_nm, 8 * (b + 1))
                if b >= 2:
                    s.wait_ge(pt_, b - 1)     # tail(b-2) done reading E/esum
                s.activation(E_all[b % 2][:], NM[b % 2][:], Exp, bias=c100[:],
                             scale=-1.0, accum_out=esum[b % 2][:]).then_inc(at, 1)

            for g in range(NSLOT):
                n = g - L_OC
                if 0 <= n < NTOT and not outcp_on_dve(n):
                    outcp_a(n)
                n = g - L_EX
                if 0 <= n < NTOT:
                    ex(n)
                for b in range(NB):
                    if g == 16 * b + 21:
                        t1(b)

        # ---------------- DVE ----------------
        @blk.vector
        def _(v):
            def nm_pair(pg):
                b, pq = divmod(pg, 8)
                if pq == 0 and b >= 2:
                    v.wait_ge(at, b - 1)   # T1(b-2) done reading NM[b%2]
                base = (2 * pg) % 6
                rd = v.tensor_reduce(NM[b % 2][:, 2 * pq:2 * pq + 2],
                                     pS[:, base:base + 2, :], AX.X, OP.max,
                                     negate=True)
                rd._wait_ge(pe_s, 2 * pg + 2)
                rd.then_inc(dve_nm, 1)

            def ptr_pair(p):
                n1 = 2 * p + 1
                if p >= 2:
                    v.wait_ge(pe_o, n1 - 3)   # mm2s of pair evicted 2 pairs ago
                cp = v.tensor_copy(ptr[p % 2][:],
                                   pPT[p % 2][:].bitcast(BF16)[:, 0:512])
                cp._wait_ge(pe_pt, n1 + 1)
                cp.then_inc(dve_ptr, 1)

            def recip(n):
                b, i = divmod(n, NBLK)
                if i == 0 and b >= 2:
                    v.wait_ge(act_oA, cnt_a(16 * (b - 1) - 1))   # RS[b%2] WAR
                    v.wait_ge(act_oD, cnt_d(16 * (b - 1) - 1))
                rc = v.reciprocal(RS[b % 2][:, i:i + 1], psum_col(n))
                rc._wait_ge(pe_o, n + 1)
                rc.then_inc(dve_rs, 1)

            def outcp_d(n):
                b, i = divmod(n, NBLK)
                ko = n % 3
                v.wait_ge(dve_rs, n + 1)
                v.tensor_scalar_mul(o_sb[b % 2][:, i, :], pO(ko),
                                    RS[b % 2][:, i:i + 1]).then_inc(act_oD, 1)

            def qc_stage(b):
                v.wait_ge(pt_, b + 1)
                if b >= 2:
                    v.wait_ge(s_qc, 16 * (b - 1))    # qc DMA(b-2) done
                v.tensor_copy(qc_sb[b % 2][:, 0:2], pM[:, 300:302])
                v.tensor_copy(qc_sb[b % 2][0:1, 2:3],
                              pM[0:1, 310:311]).then_inc(dv_qc, 1)

            for g in range(NSLOT):
                if g >= 3 and (g - 3) % 2 == 0 and (g - 3) // 2 < NTOT // 2:
                    nm_pair((g - 3) // 2)
                if g >= 9 and g % 2 == 1 and (g - 9) // 2 < NTOT // 2:
                    ptr_pair((g - 9) // 2)
                n = g - L_RC
                if 0 <= n < NTOT:
                    recip(n)
                n = g - L_OC
                if 0 <= n < NTOT and outcp_on_dve(n):
                    outcp_d(n)
                for b in range(NB):
                    if g == 16 * b + 25:
                        qc_stage(b)

    return nc, es


_CACHE = {}


def _get_program():
    if "nc" not in _CACHE:
        nc, es = build_program()
        _CACHE["nc"] = nc
        _CACHE["es"] = es
    return _CACHE["nc"]


def kernel(context_repr, question_repr, context_len, question_len):
    C = np.ascontiguousarray(np.asarray(context_repr, np.float32))
    Q = np.ascontiguousarray(np.asarray(question_repr, np.float32))
    context_len = np.asarray(context_len, np.int32)
    question_len = np.asarray(question_len, np.int32)
    bf16 = ml_dtypes.bfloat16

    cm = (np.arange(TC)[None, :] < context_len[:, None]).astype(np.float32)
    qm = (np.arange(TQ)[None, :] < question_len[:, None]).astype(np.float32)
    mcf = np.stack([SQ * cm, np.ones_like(cm)], axis=1)
    mqf = np.stack([SQ * qm, np.full_like(qm, NEG)], axis=1)
    mskh = np.ascontiguousarray(np.concatenate([mqf, mcf], axis=2))

    ct = C.transpose(0, 2, 1).reshape(B, 2, 128, TC)
    qt = Q.transpose(0, 2, 1).reshape(B, 2, 128, TQ)
    ctq = np.ascontiguousarray(np.concatenate([qt, ct], axis=3))
    cn = C.astype(bf16)
    qn = np.concatenate([Q, np.ones((B, TQ, 1), np.float32)], axis=2)
    qn = np.ascontiguousarray(qn.reshape(B, 2, 128, QN_W).astype(bf16))
    identb = np.eye(128, dtype=bf16)
    c100 = np.full((128, 1), -100.0, np.float32)
    ones128 = np.ones((128, 1), np.float32)

    nc = _get_program()
    in_maps = []
    for core in range(NCORES):
        sl = slice(core * NB, (core + 1) * NB)
        in_maps.append({
            "ctq": np.ascontiguousarray(ctq[sl]),
            "cn": np.ascontiguousarray(cn[sl]),
            "qn": np.ascontiguousarray(qn[sl]),
            "msk": np.ascontiguousarray(mskh[sl]),
            "identb": identb,
            "c100": c100,
            "ones128": ones128,
        })

    res = run_bass_kernel_spmd(nc, in_maps, list(range(NCORES)))
    out1 = np.concatenate(
        [np.asarray(r["o"]).reshape(NB, TC, D).astype(np.float32)
         for r in res.results], axis=0)
    qc_raw = np.concatenate(
        [np.asarray(r["qc"]).reshape(NB, 128, 3) for r in res.results], axis=0)
    q2c = qc_raw[:, :, 0:2].transpose(0, 2, 1).reshape(B, D) / qc_raw[:, 0:1, 2]
    out2 = np.ascontiguousarray(np.broadcast_to(q2c[:, None, :], (B, TC, D)))
    return out1, out2


# revision 32
# speedup vs baseline: 1.2139x; 1.2139x over previous
"""BiAttention TRN2 kernel: data-parallel over batch across 8 NeuronCores.

Self-contained: hardcodes B=32, Tc=2048, Tq=256, D=256, 8 cores, 4 batches/core.

Design:
- Host pre-transposes C and Q: device receives C^T f32 (sim lhsT), C bf16
  (q2c lhsT), Q^T f32 (sim rhs), Q bf16 with a ones column (mm2 rhs).
  No C/Q transposes or PSUM->SBUF staging on PE/DVE.
- Row sums come free from the ones column of the mm2 rhs (out col 256),
  so exp needs no accumulator read.
- q2c computed as 1-row-moving matmuls (out [128,1]) -- near-zero PE cost;
  normalization by the total happens on host.
- Output stored bf16 (halves O DMA); normalize-muls split 3:1 ACT/DVE;
  row-max quad reductions on DVE; all DMAs issued from SP HWDGE queues.
- All engine threads run one continuous software-pipelined stream over the
  64 global blocks (no per-batch barriers); deep rings (pS 6 slots, p_sb 6,
  dual P^T PSUM banks, pO ring 3) keep cross-engine feedback loops slack.
"""
import numpy as np
import ml_dtypes

import concourse.bass as bass
from concourse import mybir
from concourse.bass_utils import run_bass_kernel_spmd

F32 = mybir.dt.float32
F32R = mybir.dt.float32r
BF16 = mybir.dt.bfloat16
F16 = mybir.dt.float16
Exp = mybir.ActivationFunctionType.Exp
AX = mybir.AxisListType
OP = mybir.AluOpType

B, TC, TQ, D = 32, 2048, 256, 256
NCORES = 8
NB = B // NCORES          # batches per core = 4
NBLK = TC // 128          # c-blocks per batch = 16
NTOT = NB * NBLK          # total blocks = 64
NEG = -(2.0 ** 96)
SQ = 2.0 ** 48
QN_W = TQ + 1             # mm2 rhs width: D cols of Q + ones column

# pipeline stage lags (in global slots)
L_EX = 5
L_PT = 7
L_MM = 10
L_RC = 12
L_OC = 13
NSLOT = NTOT + L_OC + 2


def outcp_on_dve(n):
    return n % 8 in (2, 5, 7)


def cnt_a(m):
    """# of outcp indices 0..m handled by ACT."""
    return sum(1 for j in range(m + 1) if not outcp_on_dve(j))


def cnt_d(m):
    """# of outcp indices 0..m handled by DVE."""
    return sum(1 for j in range(m + 1) if outcp_on_dve(j))


def build_program():
    nc = bass.Bass()
    ctq_d = nc.declare_dram_parameter("ctq", [NB, 2, 128, TQ + TC], F16,
                                      isOutput=False)
    cn_d = nc.declare_dram_parameter("cn", [NB, TC, D], BF16, isOutput=False)
    qn_d = nc.declare_dram_parameter("qn", [NB, 2, 128, QN_W], BF16, isOutput=False)
    msk_d = nc.declare_dram_parameter("msk", [NB, 2, TC + TQ], F32R, isOutput=False)
    id_d = nc.declare_dram_parameter("identb", [128, 128], BF16, isOutput=False)
    c100_d = nc.declare_dram_parameter("c100", [128, 1], F32, isOutput=False)
    ones_d = nc.declare_dram_parameter("ones128", [128, 1], F32, isOutput=False)

    o_d = nc.declare_dram_parameter("o", [NB, TC, D], BF16, isOutput=True)
    qc_d = nc.declare_dram_parameter("qc", [NB, 128, 3], F32, isOutput=True)

    from contextlib import ExitStack
    es = ExitStack()
    _ctr = [0]

    def sb(shape, dt, name=None):
        _ctr[0] += 1
        return es.enter_context(nc.sbuf_tensor(name or f"sb{_ctr[0]}", shape, dt))

    def ps(shape, dt, name=None):
        _ctr[0] += 1
        return es.enter_context(nc.psum_tensor(name or f"ps{_ctr[0]}", shape, dt))

    def sem(name):
        return es.enter_context(nc.semaphore(name))

    # ---- SBUF ----
    # merged [Q^T | C^T] per batch: cols 0:TQ = Q^T, TQ: = C^T (fp16)
    ctq = [sb([128, 2, TQ + TC], F16) for _ in range(3)]
    cbn = [sb([128, NBLK, D], BF16) for _ in range(3)]  # C natural bf16
    qnb = [sb([128, 2, QN_W], BF16) for _ in range(3)]  # Q nat + ones col
    msk = [sb([2, TC + TQ], F32R) for _ in range(3)]    # [c-mask | q-mask] features
    identb = sb([128, 128], BF16)
    c100 = sb([128, 1], F32)                            # bias constant -100
    ones128 = sb([128, 1], F32)
    p_sb = [sb([128, TQ], BF16) for _ in range(6)]      # exp(S-m) (bf16), 6-deep
    ptr = [sb([128, 2, 2, 128], BF16) for _ in range(2)]  # P^T (q, blkpar, qhalf, c)
    NM = [sb([128, NBLK], F32) for _ in range(NB)]      # -rowmax per block column
    RS = [sb([128, NBLK], F32) for _ in range(NB)]      # 1/rowsum
    E_all = [sb([128, NBLK], BF16) for _ in range(NB)]  # exp(m - 100) for q2c
    esum = [sb([128, 1], F32) for _ in range(NB)]
    o_sb = [sb([128, NBLK, D], BF16) for _ in range(3)]  # output batch buffer
    qc_sb = [sb([128, 3], F32) for _ in range(2)]       # staged q2cT + total

    # ---- PSUM (8 banks) ----
    pS = ps([128, 6, 256], F32)       # sim ring, 6 slots (3 banks)
    # P^T pair banks: lower half (f32 cols 0:256) holds bf16 P^T pairs via
    # bitcast; upper half of bank 1 doubles as the q2c accumulator region.
    pPT = [ps([128, 512], F32) for _ in range(2)]
    pOb = [ps([128, QN_W], F32) for _ in range(3)]   # mm2 out (+rowsum col)
    pM = pPT[1]                       # q2cT cols 300:302, total at [0:1, 310:311]

    def pO(ko):
        return pOb[ko][:, 0:256]

    def psum_col(n):
        return pOb[n % 3][:, 256:257]

    sems = {}
    for name in ("s_out", "s_qc", "pe_s", "pe_pt", "pe_o", "pt_",
                 "dve_nm", "act_p", "act_oA", "act_oD", "dve_ptr", "dve_rs",
                 "at", "dv_qc"):
        sems[name] = sem(name)
    IN_TAGS = ["msk", "ctq0", "ctq1", "ctq2", "ctq3", "ctq4", "qnb", "cbn",
               "const"]
    s_in = {t: sem("s_" + t) for t in IN_TAGS}
    s_out = sems["s_out"]; s_qc = sems["s_qc"]
    pe_s = sems["pe_s"]; pe_pt = sems["pe_pt"]; pe_o = sems["pe_o"]
    pt_ = sems["pt_"]; dve_nm = sems["dve_nm"]; act_p = sems["act_p"]
    act_oA = sems["act_oA"]; act_oD = sems["act_oD"]
    dve_ptr = sems["dve_ptr"]; dve_rs = sems["dve_rs"]; at = sems["at"]
    dv_qc = sems["dv_qc"]

    # Input DMA schedule: per batch, sim-critical tensors first, C^T in
    # 4 column-quarters so early blocks can start before the full load.
    # Consts are interleaved after batch 0's sim-critical loads.
    CTQ_CUTS = [0, TQ + 128, TQ + 128 * 5, TQ + 128 * 9, TQ + 128 * 13,
                TQ + TC]
    NCHUNK = len(CTQ_CUTS) - 1
    TH_I = {0: 0, 1: 1, 5: 2, 9: 3, 13: 4}   # block -> chunk it needs

    blk = es.enter_context(nc.Block())
    with blk:
        # ---------------- SP: all DMAs ----------------
        @blk.sync
        def _(sy):
            def issue_one(b, tag):
                if tag == "msk":
                    return sy.dma_start(msk[b % 3][:], msk_d[b])
                if tag.startswith("ctq"):
                    q = int(tag[3])
                    lo, hi = CTQ_CUTS[q], CTQ_CUTS[q + 1]
                    return sy.dma_start(
                        ctq[b % 3][:, :, lo:hi],
                        ctq_d[b, :, :, lo:hi].rearrange("k p c -> p k c"))
                if tag == "qnb":
                    return sy.dma_start(qnb[b % 3][:],
                                        qn_d[b].rearrange("k p d -> p k d"))
                if tag == "cbn":
                    return sy.dma_start(
                        cbn[b % 3][:],
                        cn_d[b].rearrange("(i p) d -> p i d", p=128))
                raise AssertionError(tag)

            def issue_inputs(b):
                if b >= 3:
                    # WAR: batch b-3 consumers done with the b%3 buffers
                    sy.wait_ge(pe_s, 16 * (b - 2))
                    sy.wait_ge(pe_o, 16 * (b - 2))
                    sy.wait_ge(pt_, b - 2)
                tags = ["msk"] + [f"ctq{q}" for q in range(NCHUNK)]
                tags += ["qnb", "cbn"]
                for tag in tags:
                    if b == 0 and tag in ("msk", "ctq0"):
                        continue  # issued from the ACT queue at startup
                    if b >= 1:
                        # serialize same-tag DMAs across batches so tag
                        # sem thresholds are unambiguous under unordered
                        # DMA completion
                        sy.wait_ge(s_in[tag], 16 * b)
                    issue_one(b, tag).then_inc(s_in[tag], 16)

            issue_inputs(0)
            issue_inputs(1)
            for b in range(NB):
                if b + 2 < NB:
                    issue_inputs(b + 2)
                if b >= 2:
                    sy.wait_ge(s_out, 64 * (b - 1))
                for q4 in range(4):
                    m = 16 * b + 4 * q4 + 3
                    sy.wait_ge(act_oA, cnt_a(m))
                    sy.wait_ge(act_oD, cnt_d(m))
                    sy.dma_start(
                        o_d[b, 512 * q4:512 * (q4 + 1)].rearrange(
                            "(i p) d -> p i d", p=128),
                        o_sb[b % 3][:, 4 * q4:4 * (q4 + 1), :]).then_inc(s_out, 16)
                sy.wait_ge(dv_qc, b + 1)
                sy.dma_start(qc_d[b], qc_sb[b % 2][:]).then_inc(s_qc, 16)

        # ---------------- PE ----------------
        @blk.tensor
        def _(t):
            def sim(n):
                b, i = divmod(n, NBLK)
                sl = n % 6
                if i == 0:
                    t.wait_ge(s_in["msk"], 16 * (b + 1))
                if i in TH_I:
                    t.wait_ge(s_in[f"ctq{TH_I[i]}"], 16 * (b + 1))
                if n >= 6:
                    t.wait_ge(act_p, n - 5)   # exp(n-6) done -> pS slot free
                t.matmul(pS[:, sl, :],
                         msk[b % 3][:, TQ + 128 * i:TQ + 128 * (i + 1)],
                         msk[b % 3][:, 0:TQ], start=True, stop=False)
                t.matmul(pS[:, sl, :],
                         ctq[b % 3][:, 0, TQ + 128 * i:TQ + 128 * (i + 1)],
                         ctq[b % 3][:, 0, 0:TQ], start=False, stop=False)
                t.matmul(pS[:, sl, :],
                         ctq[b % 3][:, 1, TQ + 128 * i:TQ + 128 * (i + 1)],
                         ctq[b % 3][:, 1, 0:TQ], start=False,
                         stop=True).then_inc(pe_s, 1)

            def pt_tr(n):
                k = n % 2
                pb = (n // 2) % 2
                if n >= 4:
                    t.wait_ge(dve_ptr, n // 2 - 1)   # pPT[pb] prior pair copied
                if n == 0:
                    t.wait_ge(s_in["const"], 48)
                ptb = pPT[pb][:].bitcast(BF16)
                tr0 = t.transpose(ptb[:, k * 256:k * 256 + 128],
                                  p_sb[n % 6][:, 0:128], identb[:])
                tr0._wait_ge(act_p, n + 1)
                t.transpose(ptb[:, k * 256 + 128:k * 256 + 256],
                            p_sb[n % 6][:, 128:256], identb[:]).then_inc(pe_pt, 1)

            def mm2(n):
                b, i = divmod(n, NBLK)
                ko = n % 3
                pp = (n // 2) % 2
                if i == 0:
                    t.wait_ge(s_in["qnb"], 16 * (b + 1))
                if n >= 3:
                    m = n - 3
                    t.wait_ge(act_oA, cnt_a(m))    # outcp(n-3) done
                    t.wait_ge(act_oD, cnt_d(m))
                    t.wait_ge(dve_rs, n - 2)       # recip(n-3) done
                mm0 = t.matmul(pOb[ko][:], ptr[pp][:, n % 2, 0],
                               qnb[b % 3][:, 0, :], start=True, stop=False)
                mm0._wait_ge(dve_ptr, n // 2 + 1)
                t.matmul(pOb[ko][:], ptr[pp][:, n % 2, 1], qnb[b % 3][:, 1, :],
                         start=False, stop=True).then_inc(pe_o, 1)

            def tail(b):
                t.wait_ge(s_in["cbn"], 16 * (b + 1))
                t.wait_ge(at, b + 1)          # E_all/esum ready
                if b >= 1:
                    t.wait_ge(dv_qc, b)       # qc staging of b-1 done (pM free)
                for dh in range(2):
                    for i in range(NBLK):
                        t.matmul(pM[:, 300 + dh:301 + dh],
                                 cbn[b % 3][:, i, 128 * dh:128 * (dh + 1)],
                                 E_all[b][:, i:i + 1],
                                 start=(i == 0), stop=(i == NBLK - 1))
                t.matmul(pM[0:1, 310:311], esum[b][:], ones128[:],
                         start=True, stop=True).then_inc(pt_, 1)

            for g in range(NSLOT):
                n = g
                if 0 <= n < NTOT:
                    sim(n)
                n = g - L_PT
                if 0 <= n < NTOT:
                    pt_tr(n)
                n = g - L_MM
                if 0 <= n < NTOT:
                    mm2(n)
                for b in range(NB):
                    if g == 16 * b + 23:
                        tail(b)

        # ---------------- ACT ----------------
        @blk.scalar
        def _(s):
            def ex(n):
                b, i = divmod(n, NBLK)
                sl = n % 6
                if n >= 6:
                    s.wait_ge(pe_pt, n - 5)   # p_sb 6-deep WAR
                ac = s.activation(p_sb[n % 6][:], pS[:, sl, :], Exp,
                                  bias=NM[b][:, i:i + 1])
                ac._wait_ge(dve_nm, 8 * b + i // 2 + 1)
                ac.then_inc(act_p, 1)

            def outcp_a(n):
                b, i = divmod(n, NBLK)
                ko = n % 3
                s.wait_ge(dve_rs, n + 1)
                if i == 0 and b >= 3:
                    s.wait_ge(s_out, 64 * (b - 2))
                s.mul(o_sb[b % 3][:, i, :], pO(ko),
                      RS[b][:, i:i + 1]).then_inc(act_oA, 1)

            def t1(b):
                if b == 0:
                    s.wait_ge(s_in["const"], 48)
                s.wait_ge(dve_nm, 8 * (b + 1))
                s.activation(E_all[b][:], NM[b][:], Exp, bias=c100[:],
                             scale=-1.0, accum_out=esum[b][:]).then_inc(at, 1)

            s.dma_start(msk[0][:], msk_d[0]).then_inc(s_in["msk"], 16)
            s.dma_start(
                ctq[0][:, :, CTQ_CUTS[0]:CTQ_CUTS[1]],
                ctq_d[0, :, :, CTQ_CUTS[0]:CTQ_CUTS[1]].rearrange(
                    "k p c -> p k c")).then_inc(s_in["ctq0"], 16)
            s.dma_start(identb[:], id_d[:]).then_inc(s_in["const"], 16)
            s.dma_start(c100[:], c100_d[:]).then_inc(s_in["const"], 16)
            s.dma_start(ones128[:], ones_d[:]).then_inc(s_in["const"], 16)
            for g in range(NSLOT):
                n = g - L_EX
                if 0 <= n < NTOT:
                    ex(n)
                n = g - L_OC
                if 0 <= n < NTOT and not outcp_on_dve(n):
                    outcp_a(n)
                for b in range(NB):
                    if g == 16 * b + 21:
                        t1(b)

        # ---------------- DVE ----------------
        @blk.vector
        def _(v):
            def nm_pair(pg):
                b, pq = divmod(pg, 8)
                if pq == 0 and b >= 2:
                    v.wait_ge(at, b - 1)   # T1(b-2) done reading NM[b%2]
                base = (2 * pg) % 6
                rd = v.tensor_reduce(NM[b][:, 2 * pq:2 * pq + 2],
                                     pS[:, base:base + 2, :], AX.X, OP.max,
                                     negate=True)
                rd._wait_ge(pe_s, 2 * pg + 2)
                rd.then_inc(dve_nm, 1)

            def ptr_pair(p):
                n1 = 2 * p + 1
                if p >= 2:
                    v.wait_ge(pe_o, n1 - 3)   # mm2s of pair evicted 2 pairs ago
                cp = v.tensor_copy(ptr[p % 2][:],
                                   pPT[p % 2][:].bitcast(BF16)[:, 0:512])
                cp._wait_ge(pe_pt, n1 + 1)
                cp.then_inc(dve_ptr, 1)

            def recip(n):
                b, i = divmod(n, NBLK)
                if i == 0 and b >= 2:
                    v.wait_ge(act_oA, cnt_a(16 * (b - 1) - 1))   # RS[b%2] WAR
                    v.wait_ge(act_oD, cnt_d(16 * (b - 1) - 1))
                rc = v.reciprocal(RS[b][:, i:i + 1], psum_col(n))
                rc._wait_ge(pe_o, n + 1)
                rc.then_inc(dve_rs, 1)

            def outcp_d(n):
                b, i = divmod(n, NBLK)
                ko = n % 3
                v.wait_ge(dve_rs, n + 1)
                v.tensor_scalar_mul(o_sb[b % 3][:, i, :], pO(ko),
                                    RS[b][:, i:i + 1]).then_inc(act_oD, 1)

            def qc_stage(b):
                v.wait_ge(pt_, b + 1)
                if b >= 2:
                    v.wait_ge(s_qc, 16 * (b - 1))    # qc DMA(b-2) done
                v.tensor_copy(qc_sb[b % 2][:, 0:2], pM[:, 300:302])
                v.tensor_copy(qc_sb[b % 2][0:1, 2:3],
                              pM[0:1, 310:311]).then_inc(dv_qc, 1)

            for g in range(NSLOT):
                if g >= 3 and (g - 3) % 2 == 0 and (g - 3) // 2 < NTOT // 2:
                    nm_pair((g - 3) // 2)
                if g >= 9 and g % 2 == 1 and (g - 9) // 2 < NTOT // 2:
                    ptr_pair((g - 9) // 2)
                n = g - L_RC
                if 0 <= n < NTOT:
                    recip(n)
                n = g - L_OC
                if 0 <= n < NTOT and outcp_on_dve(n):
                    outcp_d(n)
                for b in range(NB):
                    if g == 16 * b + 25:
                        qc_stage(b)

    return nc, es


_CACHE = {}


def _get_program():
    if "nc" not in _CACHE:
        nc, es = build_program()
        _CACHE["nc"] = nc
        _CACHE["es"] = es
    return _CACHE["nc"]


def kernel(context_repr, question_repr, context_len, question_len):
    C = np.ascontiguousarray(np.asarray(context_repr, np.float32))
    Q = np.ascontiguousarray(np.asarray(question_repr, np.float32))
    context_len = np.asarray(context_len, np.int32)
    question_len = np.asarray(question_len, np.int32)
    bf16 = ml_dtypes.bfloat16

    cm = (np.arange(TC)[None, :] < context_len[:, None]).astype(np.float32)
    qm = (np.arange(TQ)[None, :] < question_len[:, None]).astype(np.float32)
    mcf = np.stack([SQ * cm, np.ones_like(cm)], axis=1)
    mqf = np.stack([SQ * qm, np.full_like(qm, NEG)], axis=1)
    mskh = np.ascontiguousarray(np.concatenate([mqf, mcf], axis=2))

    ct = C.transpose(0, 2, 1).reshape(B, 2, 128, TC)
    qt = Q.transpose(0, 2, 1).reshape(B, 2, 128, TQ)
    ctq = np.ascontiguousarray(
        np.concatenate([qt, ct], axis=3).astype(np.float16))
    cn = C.astype(bf16)
    qn = np.concatenate([Q, np.ones((B, TQ, 1), np.float32)], axis=2)
    qn = np.ascontiguousarray(qn.reshape(B, 2, 128, QN_W).astype(bf16))
    identb = np.eye(128, dtype=bf16)
    c100 = np.full((128, 1), -100.0, np.float32)
    ones128 = np.ones((128, 1), np.float32)

    nc = _get_program()
    in_maps = []
    for core in range(NCORES):
        sl = slice(core * NB, (core + 1) * NB)
        in_maps.append({
            "ctq": np.ascontiguousarray(ctq[sl]),
            "cn": np.ascontiguousarray(cn[sl]),
            "qn": np.ascontiguousarray(qn[sl]),
            "msk": np.ascontiguousarray(mskh[sl]),
            "identb": identb,
            "c100": c100,
            "ones128": ones128,
        })

    res = run_bass_kernel_spmd(nc, in_maps, list(range(NCORES)))
    out1 = np.concatenate(
        [np.asarray(r["o"]).reshape(NB, TC, D).astype(np.float32)
         for r in res.results], axis=0)
    qc_raw = np.concatenate(
        [np.asarray(r["qc"]).reshape(NB, 128, 3) for r in res.results], axis=0)
    q2c = qc_raw[:, :, 0:2].transpose(0, 2, 1).reshape(B, D) / qc_raw[:, 0:1, 2]
    out2 = np.ascontiguousarray(np.broadcast_to(q2c[:, None, :], (B, TC, D)))
    return out1, out2


# revision 45
# speedup vs baseline: 1.2145x; 1.0005x over previous
"""BiAttention TRN2 kernel: data-parallel over batch across 8 NeuronCores.

Self-contained: hardcodes B=32, Tc=2048, Tq=256, D=256, 8 cores, 4 batches/core.

Design:
- Host pre-transposes C and Q: device receives C^T f32 (sim lhsT), C bf16
  (q2c lhsT), Q^T f32 (sim rhs), Q bf16 with a ones column (mm2 rhs).
  No C/Q transposes or PSUM->SBUF staging on PE/DVE.
- Row sums come free from the ones column of the mm2 rhs (out col 256),
  so exp needs no accumulator read.
- q2c computed as 1-row-moving matmuls (out [128,1]) -- near-zero PE cost;
  normalization by the total happens on host.
- Output stored bf16 (halves O DMA); normalize-muls split 3:1 ACT/DVE;
  row-max quad reductions on DVE; all DMAs issued from SP HWDGE queues.
- All engine threads run one continuous software-pipelined stream over the
  64 global blocks (no per-batch barriers); deep rings (pS 6 slots, p_sb 6,
  dual P^T PSUM banks, pO ring 3) keep cross-engine feedback loops slack.
"""
import numpy as np
import ml_dtypes

import concourse.bass as bass
from concourse import mybir
from concourse.bass_utils import run_bass_kernel_spmd

F32 = mybir.dt.float32
F32R = mybir.dt.float32r
BF16 = mybir.dt.bfloat16
F16 = mybir.dt.float16
Exp = mybir.ActivationFunctionType.Exp
AX = mybir.AxisListType
OP = mybir.AluOpType

B, TC, TQ, D = 32, 2048, 256, 256
NCORES = 8
NB = B // NCORES          # batches per core = 4
NBLK = TC // 128          # c-blocks per batch = 16
NTOT = NB * NBLK          # total blocks = 64
NEG = -(2.0 ** 96)
SQ = 2.0 ** 48
QN_W = TQ + 1             # mm2 rhs width: D cols of Q + ones column

# pipeline stage lags (in global slots)
L_EX = 5
L_PT = 7
L_MM = 10
L_RC = 12
L_OC = 13
NSLOT = NTOT + L_OC + 2


def outcp_on_dve(n):
    return n % 8 in (2, 5, 7)


def cnt_a(m):
    """# of outcp indices 0..m handled by ACT."""
    return sum(1 for j in range(m + 1) if not outcp_on_dve(j))


def cnt_d(m):
    """# of outcp indices 0..m handled by DVE."""
    return sum(1 for j in range(m + 1) if outcp_on_dve(j))


def build_program():
    nc = bass.Bass()
    ctq_d = nc.declare_dram_parameter("ctq", [NB, 2, 128, TQ + TC], F16,
                                      isOutput=False)
    cn_d = nc.declare_dram_parameter("cn", [NB, TC, D], BF16, isOutput=False)
    qn_d = nc.declare_dram_parameter("qn", [NB, 2, 128, QN_W], BF16, isOutput=False)
    msk_d = nc.declare_dram_parameter("msk", [NB, 2, TC + TQ], F32R, isOutput=False)
    id_d = nc.declare_dram_parameter("identb", [128, 128], BF16, isOutput=False)
    c100_d = nc.declare_dram_parameter("c100", [128, 1], F32, isOutput=False)
    ones_d = nc.declare_dram_parameter("ones128", [128, 1], F32, isOutput=False)

    o_d = nc.declare_dram_parameter("o", [NB, TC, D], BF16, isOutput=True)
    qc_d = nc.declare_dram_parameter("qc", [NB, 128, 3], F32, isOutput=True)

    from contextlib import ExitStack
    es = ExitStack()
    _ctr = [0]

    def sb(shape, dt, name=None):
        _ctr[0] += 1
        return es.enter_context(nc.sbuf_tensor(name or f"sb{_ctr[0]}", shape, dt))

    def ps(shape, dt, name=None):
        _ctr[0] += 1
        return es.enter_context(nc.psum_tensor(name or f"ps{_ctr[0]}", shape, dt))

    def sem(name):
        return es.enter_context(nc.semaphore(name))

    # ---- SBUF ----
    # merged [Q^T | C^T] per batch: cols 0:TQ = Q^T, TQ: = C^T (fp16)
    ctq = [sb([128, 2, TQ + TC], F16) for _ in range(3)]
    cbn = [sb([128, NBLK, D], BF16) for _ in range(3)]  # C natural bf16
    qnb = [sb([128, 2, QN_W], BF16) for _ in range(3)]  # Q nat + ones col
    msk = [sb([2, TC + TQ], F32R) for _ in range(3)]    # [c-mask | q-mask] features
    identb = sb([128, 128], BF16)
    c100 = sb([128, 1], F32)                            # bias constant -100
    ones128 = sb([128, 1], F32)
    p_sb = [sb([128, TQ], BF16) for _ in range(6)]      # exp(S-m) (bf16), 6-deep
    ptr = [sb([128, 2, 2, 128], BF16) for _ in range(2)]  # P^T (q, blkpar, qhalf, c)
    NM = [sb([128, NBLK], F32) for _ in range(NB)]      # -rowmax per block column
    RS = [sb([128, NBLK], F32) for _ in range(NB)]      # 1/rowsum
    E_all = [sb([128, NBLK], BF16) for _ in range(NB)]  # exp(m - 100) for q2c
    esum = [sb([128, 1], F32) for _ in range(NB)]
    o_sb = [sb([128, NBLK, D], BF16) for _ in range(3)]  # output batch buffer
    qc_sb = [sb([128, 3], F32) for _ in range(2)]       # staged q2cT + total

    # ---- PSUM (8 banks) ----
    pS = ps([128, 6, 256], F32)       # sim ring, 6 slots (3 banks)
    # P^T pair banks: lower half (f32 cols 0:256) holds bf16 P^T pairs via
    # bitcast; upper half of bank 1 doubles as the q2c accumulator region.
    pPT = [ps([128, 512], F32) for _ in range(2)]
    pOb = [ps([128, QN_W], F32) for _ in range(3)]   # mm2 out (+rowsum col)
    pM = pPT[1]                       # q2cT cols 300:302, total at [0:1, 310:311]

    def pO(ko):
        return pOb[ko][:, 0:256]

    def psum_col(n):
        return pOb[n % 3][:, 256:257]

    sems = {}
    for name in ("s_out", "s_qc", "pe_s", "pe_pt", "pe_o", "pt_",
                 "dve_nm", "act_p", "act_oA", "act_oD", "dve_ptr", "dve_rs",
                 "at", "dv_qc"):
        sems[name] = sem(name)
    IN_TAGS = ["msk", "ctq0", "ctq1", "ctq2", "ctq3", "ctq4", "qnb", "cbn",
               "const"]
    s_in = {t: sem("s_" + t) for t in IN_TAGS}
    s_out = sems["s_out"]; s_qc = sems["s_qc"]
    pe_s = sems["pe_s"]; pe_pt = sems["pe_pt"]; pe_o = sems["pe_o"]
    pt_ = sems["pt_"]; dve_nm = sems["dve_nm"]; act_p = sems["act_p"]
    act_oA = sems["act_oA"]; act_oD = sems["act_oD"]
    dve_ptr = sems["dve_ptr"]; dve_rs = sems["dve_rs"]; at = sems["at"]
    dv_qc = sems["dv_qc"]

    # Input DMA schedule: per batch, sim-critical tensors first, C^T in
    # 4 column-quarters so early blocks can start before the full load.
    # Consts are interleaved after batch 0's sim-critical loads.
    import os as _os3
    _CV = int(_os3.environ.get("K_CUTS", 0))
    if _CV == 0:
        CTQ_CUTS = [0, TQ + 128, TQ + 128 * 5, TQ + 128 * 9, TQ + 128 * 13,
                    TQ + TC]
        TH_I = {0: 0, 1: 1, 5: 2, 9: 3, 13: 4}
    elif _CV == 1:
        CTQ_CUTS = [0, TQ + 128, TQ + 128 * 3, TQ + 128 * 6, TQ + 128 * 10,
                    TQ + TC]
        TH_I = {0: 0, 1: 1, 3: 2, 6: 3, 10: 4}
    else:
        CTQ_CUTS = [0, TQ + 256, TQ + 512, TQ + 1024, TQ + 1536, TQ + TC]
        TH_I = {0: 0, 2: 1, 4: 2, 8: 3, 12: 4}
    NCHUNK = len(CTQ_CUTS) - 1

    blk = es.enter_context(nc.Block())
    with blk:
        # ---------------- SP: all DMAs ----------------
        @blk.sync
        def _(sy):
            def issue_one(b, tag):
                if tag == "msk":
                    return sy.dma_start(msk[b % 3][:], msk_d[b])
                if tag.startswith("ctq"):
                    q = int(tag[3])
                    lo, hi = CTQ_CUTS[q], CTQ_CUTS[q + 1]
                    return sy.dma_start(
                        ctq[b % 3][:, :, lo:hi],
                        ctq_d[b, :, :, lo:hi].rearrange("k p c -> p k c"))
                if tag == "qnb":
                    return sy.dma_start(qnb[b % 3][:],
                                        qn_d[b].rearrange("k p d -> p k d"))
                if tag == "cbn":
                    return sy.dma_start(
                        cbn[b % 3][:],
                        cn_d[b].rearrange("(i p) d -> p i d", p=128))
                raise AssertionError(tag)

            def issue_inputs(b):
                if b >= 3:
                    # WAR: batch b-3 consumers done with the b%3 buffers
                    sy.wait_ge(pe_s, 16 * (b - 2))
                    sy.wait_ge(pe_o, 16 * (b - 2))
                    sy.wait_ge(pt_, b - 2)
                tags = ["msk"] + [f"ctq{q}" for q in range(NCHUNK)]
                tags += ["qnb", "cbn"]
                for tag in tags:
                    if b == 0 and tag in ("msk", "ctq0"):
                        continue  # issued from the ACT queue at startup
                    if b >= 1:
                        # serialize same-tag DMAs across batches so tag
                        # sem thresholds are unambiguous under unordered
                        # DMA completion
                        sy.wait_ge(s_in[tag], 16 * b)
                    issue_one(b, tag).then_inc(s_in[tag], 16)

            issue_inputs(0)
            issue_inputs(1)
            for b in range(NB):
                if b + 2 < NB:
                    issue_inputs(b + 2)
                if b >= 2:
                    sy.wait_ge(s_out, 64 * (b - 1))
                nq = 8 if b == NB - 1 else 4
                w = NBLK // nq
                for q4 in range(nq):
                    m = 16 * b + w * q4 + w - 1
                    sy.wait_ge(act_oA, cnt_a(m))
                    sy.wait_ge(act_oD, cnt_d(m))
                    sy.dma_start(
                        o_d[b, 128 * w * q4:128 * w * (q4 + 1)].rearrange(
                            "(i p) d -> p i d", p=128),
                        o_sb[b % 3][:, w * q4:w * (q4 + 1), :]).then_inc(s_out, 16)
                sy.wait_ge(dv_qc, b + 1)
                sy.dma_start(qc_d[b], qc_sb[b % 2][:]).then_inc(s_qc, 16)

        # ---------------- PE ----------------
        @blk.tensor
        def _(t):
            def sim(n):
                b, i = divmod(n, NBLK)
                sl = n % 6
                if i == 0:
                    t.wait_ge(s_in["msk"], 16 * (b + 1))
                if i in TH_I:
                    t.wait_ge(s_in[f"ctq{TH_I[i]}"], 16 * (b + 1))
                if n >= 6:
                    t.wait_ge(act_p, n - 5)   # exp(n-6) done -> pS slot free
                t.matmul(pS[:, sl, :],
                         msk[b % 3][:, TQ + 128 * i:TQ + 128 * (i + 1)],
                         msk[b % 3][:, 0:TQ], start=True, stop=False)
                t.matmul(pS[:, sl, :],
                         ctq[b % 3][:, 0, TQ + 128 * i:TQ + 128 * (i + 1)],
                         ctq[b % 3][:, 0, 0:TQ], start=False, stop=False)
                t.matmul(pS[:, sl, :],
                         ctq[b % 3][:, 1, TQ + 128 * i:TQ + 128 * (i + 1)],
                         ctq[b % 3][:, 1, 0:TQ], start=False,
                         stop=True).then_inc(pe_s, 1)

            def pt_tr(n):
                k = n % 2
                pb = (n // 2) % 2
                if n >= 4:
                    t.wait_ge(dve_ptr, n // 2 - 1)   # pPT[pb] prior pair copied
                if n == 0:
                    t.wait_ge(s_in["const"], 48)
                ptb = pPT[pb][:].bitcast(BF16)
                tr0 = t.transpose(ptb[:, k * 256:k * 256 + 128],
                                  p_sb[n % 6][:, 0:128], identb[:])
                tr0._wait_ge(act_p, n + 1)
                t.transpose(ptb[:, k * 256 + 128:k * 256 + 256],
                            p_sb[n % 6][:, 128:256], identb[:]).then_inc(pe_pt, 1)

            def mm2(n):
                b, i = divmod(n, NBLK)
                ko = n % 3
                pp = (n // 2) % 2
                if i == 0:
                    t.wait_ge(s_in["qnb"], 16 * (b + 1))
                if n >= 3:
                    m = n - 3
                    t.wait_ge(act_oA, cnt_a(m))    # outcp(n-3) done
                    t.wait_ge(act_oD, cnt_d(m))
                    t.wait_ge(dve_rs, n - 2)       # recip(n-3) done
                mm0 = t.matmul(pOb[ko][:], ptr[pp][:, n % 2, 0],
                               qnb[b % 3][:, 0, :], start=True, stop=False)
                mm0._wait_ge(dve_ptr, n // 2 + 1)
                t.matmul(pOb[ko][:], ptr[pp][:, n % 2, 1], qnb[b % 3][:, 1, :],
                         start=False, stop=True).then_inc(pe_o, 1)

            def tail(b):
                t.wait_ge(s_in["cbn"], 16 * (b + 1))
                t.wait_ge(at, b + 1)          # E_all/esum ready
                if b >= 1:
                    t.wait_ge(dv_qc, b)       # qc staging of b-1 done (pM free)
                for dh in range(2):
                    for i in range(NBLK):
                        t.matmul(pM[:, 300 + dh:301 + dh],
                                 cbn[b % 3][:, i, 128 * dh:128 * (dh + 1)],
                                 E_all[b][:, i:i + 1],
                                 start=(i == 0), stop=(i == NBLK - 1))
                t.matmul(pM[0:1, 310:311], esum[b][:], ones128[:],
                         start=True, stop=True).then_inc(pt_, 1)

            for g in range(NSLOT):
                n = g
                if 0 <= n < NTOT:
                    sim(n)
                n = g - L_PT
                if 0 <= n < NTOT:
                    pt_tr(n)
                n = g - L_MM
                if 0 <= n < NTOT:
                    mm2(n)
                for b in range(NB):
                    if g == 16 * b + TAILSLOT:
                        tail(b)

        # ---------------- ACT ----------------
        @blk.scalar
        def _(s):
            def ex(n):
                b, i = divmod(n, NBLK)
                sl = n % 6
                if n >= 6:
                    s.wait_ge(pe_pt, n - 5)   # p_sb 6-deep WAR
                ac = s.activation(p_sb[n % 6][:], pS[:, sl, :], Exp,
                                  bias=NM[b][:, i:i + 1])
                ac._wait_ge(dve_nm, 8 * b + i // 2 + 1)
                ac.then_inc(act_p, 1)

            def outcp_a(n):
                b, i = divmod(n, NBLK)
                ko = n % 3
                s.wait_ge(dve_rs, n + 1)
                if i == 0 and b >= 3:
                    s.wait_ge(s_out, 64 * (b - 2))
                s.mul(o_sb[b % 3][:, i, :], pO(ko),
                      RS[b][:, i:i + 1]).then_inc(act_oA, 1)

            def t1(b):
                if b == 0:
                    s.wait_ge(s_in["const"], 48)
                s.wait_ge(dve_nm, 8 * (b + 1))
                s.activation(E_all[b][:], NM[b][:], Exp, bias=c100[:],
                             scale=-1.0, accum_out=esum[b][:]).then_inc(at, 1)

            s.dma_start(msk[0][:], msk_d[0]).then_inc(s_in["msk"], 16)
            s.dma_start(
                ctq[0][:, :, CTQ_CUTS[0]:CTQ_CUTS[1]],
                ctq_d[0, :, :, CTQ_CUTS[0]:CTQ_CUTS[1]].rearrange(
                    "k p c -> p k c")).then_inc(s_in["ctq0"], 16)
            s.dma_start(identb[:], id_d[:]).then_inc(s_in["const"], 16)
            s.dma_start(c100[:], c100_d[:]).then_inc(s_in["const"], 16)
            s.dma_start(ones128[:], ones_d[:]).then_inc(s_in["const"], 16)
            for g in range(NSLOT):
                n = g - L_EX
                if 0 <= n < NTOT:
                    ex(n)
                n = g - L_OC
                if 0 <= n < NTOT and not outcp_on_dve(n):
                    outcp_a(n)
                for b in range(NB):
                    if g == 16 * b + 21:
                        t1(b)

        # ---------------- DVE ----------------
        @blk.vector
        def _(v):
            def nm_pair(pg):
                b, pq = divmod(pg, 8)
                if pq == 0 and b >= 2:
                    v.wait_ge(at, b - 1)   # T1(b-2) done reading NM[b%2]
                base = (2 * pg) % 6
                rd = v.tensor_reduce(NM[b][:, 2 * pq:2 * pq + 2],
                                     pS[:, base:base + 2, :], AX.X, OP.max,
                                     negate=True)
                rd._wait_ge(pe_s, 2 * pg + 2)
                rd.then_inc(dve_nm, 1)

            def ptr_pair(p):
                n1 = 2 * p + 1
                if p >= 2:
                    v.wait_ge(pe_o, n1 - 3)   # mm2s of pair evicted 2 pairs ago
                cp = v.tensor_copy(ptr[p % 2][:],
                                   pPT[p % 2][:].bitcast(BF16)[:, 0:512])
                cp._wait_ge(pe_pt, n1 + 1)
                cp.then_inc(dve_ptr, 1)

            def recip(n):
                b, i = divmod(n, NBLK)
                if i == 0 and b >= 2:
                    v.wait_ge(act_oA, cnt_a(16 * (b - 1) - 1))   # RS[b%2] WAR
                    v.wait_ge(act_oD, cnt_d(16 * (b - 1) - 1))
                rc = v.reciprocal(RS[b][:, i:i + 1], psum_col(n))
                rc._wait_ge(pe_o, n + 1)
                rc.then_inc(dve_rs, 1)

            def outcp_d(n):
                b, i = divmod(n, NBLK)
                ko = n % 3
                v.wait_ge(dve_rs, n + 1)
                v.tensor_scalar_mul(o_sb[b % 3][:, i, :], pO(ko),
                                    RS[b][:, i:i + 1]).then_inc(act_oD, 1)

            def qc_stage(b):
                v.wait_ge(pt_, b + 1)
                if b >= 2:
                    v.wait_ge(s_qc, 16 * (b - 1))    # qc DMA(b-2) done
                v.tensor_copy(qc_sb[b % 2][:, 0:2], pM[:, 300:302])
                v.tensor_copy(qc_sb[b % 2][0:1, 2:3],
                              pM[0:1, 310:311]).then_inc(dv_qc, 1)

            for g in range(NSLOT):
                if g >= 3 and (g - 3) % 2 == 0 and (g - 3) // 2 < NTOT // 2:
                    nm_pair((g - 3) // 2)
                if g >= 9 and g % 2 == 1 and (g - 9) // 2 < NTOT // 2:
                    ptr_pair((g - 9) // 2)
                n = g - L_RC
                if 0 <= n < NTOT:
                    recip(n)
                n = g - L_OC
                if 0 <= n < NTOT and outcp_on_dve(n):
                    outcp_d(n)
                for b in range(NB):
                    if g == 16 * b + 25:
                        qc_stage(b)

    return nc, es


_CACHE = {}


def _get_program():
    if "nc" not in _CACHE:
        nc, es = build_program()
        _CACHE["nc"] = nc
        _CACHE["es"] = es
    return _CACHE["nc"]


def kernel(context_repr, question_repr, context_len, question_len):
    C = np.ascontiguousarray(np.asarray(context_repr, np.float32))
    Q = np.ascontiguousarray(np.asarray(question_repr, np.float32))
    context_len = np.asarray(context_len, np.int32)
    question_len = np.asarray(question_len, np.int32)
    bf16 = ml_dtypes.bfloat16

    cm = (np.arange(TC)[None, :] < context_len[:, None]).astype(np.float32)
    qm = (np.arange(TQ)[None, :] < question_len[:, None]).astype(np.float32)
    mcf = np.stack([SQ * cm, np.ones_like(cm)], axis=1)
    mqf = np.stack([SQ * qm, np.full_like(qm, NEG)], axis=1)
    mskh = np.ascontiguousarray(np.concatenate([mqf, mcf], axis=2))

    ct = C.transpose(0, 2, 1).reshape(B, 2, 128, TC)
    qt = Q.transpose(0, 2, 1).reshape(B, 2, 128, TQ)
    ctq = np.ascontiguousarray(
        np.concatenate([qt, ct], axis=3).astype(np.float16))
    cn = C.astype(bf16)
    qn = np.concatenate([Q, np.ones((B, TQ, 1), np.float32)], axis=2)
    qn = np.ascontiguousarray(qn.reshape(B, 2, 128, QN_W).astype(bf16))
    identb = np.eye(128, dtype=bf16)
    c100 = np.full((128, 1), -100.0, np.float32)
    ones128 = np.ones((128, 1), np.float32)

    nc = _get_program()
    in_maps = []
    for core in range(NCORES):
        sl = slice(core * NB, (core + 1) * NB)
        in_maps.append({
            "ctq": np.ascontiguousarray(ctq[sl]),
            "cn": np.ascontiguousarray(cn[sl]),
            "qn": np.ascontiguousarray(qn[sl]),
            "msk": np.ascontiguousarray(mskh[sl]),
            "identb": identb,
            "c100": c100,
            "ones128": ones128,
        })

    res = run_bass_kernel_spmd(nc, in_maps, list(range(NCORES)))
    out1 = np.concatenate(
        [np.asarray(r["o"]).reshape(NB, TC, D).astype(np.float32)
         for r in res.results], axis=0)
    qc_raw = np.concatenate(
        [np.asarray(r["qc"]).reshape(NB, 128, 3) for r in res.results], axis=0)
    q2c = qc_raw[:, :, 0:2].transpose(0, 2, 1).reshape(B, D) / qc_raw[:, 0:1, 2]
    out2 = np.ascontiguousarray(np.broadcast_to(q2c[:, None, :], (B, TC, D)))
    return out1, out2
